# revision 3
# baseline (speedup 1.0000x reference)
"""Trainium2 Bass kernel for nn_CDistLoss (retrieval_knn).

Math reduction (validated against the reference to ~3e-7 rel err):
  With MARGIN=0 the relu kills every disagree term, so
    out[i] = (1/(N-1)) * sum_{j in class(i), j!=i} D_ij * (0.1+fd_j)/(0.1+fa_j)
  where fa_j = A_j/S_a, fd_j = B_j/S_d, A_j = rank of j among same-class
  distances, B_j = R_j - A_j with R_j the global rank of D_ij in row i,
  S_a = n_a*N - sum_j R_j, S_d = n_d*N - N(N-1)/2 + sum_j R_j.
  The sample_performance/min/weight factor is 1.0 to ~4e-7 in f32 and is
  dropped.

Device work per row: the [N] row of squared distances (PE fp32 matmul into
PSUM) and one count-below-threshold per same-class member (DVE is_le+accum
and ACT Sign+accum instructions, split to balance both engines). Everything
that only touches the ~64 same-class values per row (thresholds, agree
ranks, score coefficients, masks) is precomputed on the host in f32 and fed
as input tensors, which also keeps the program identical across the 8 cores.

Rows are dealt to 32 bins of 128 in class-size-descending order; bin k runs
as block k//8 on core k%8, so every core executes the same static program
with per-tier slot counts M_t.
"""

import numpy as np

N = 4096
F = 128
NCORES = 8
RPC = 512          # rows per core
NB = 4             # blocks (tiers) per core
BLK = 128          # rows per block

_cache = {}


def _host_layout(x, y):
    """Class-sorted stream layout + all host-side per-slot tensors."""
    x = np.asarray(x, dtype=np.float32)
    y = np.asarray(y).astype(np.int64)

    classes, first_idx = np.unique(y, return_index=True)
    members = {c: np.where(y == c)[0] for c in classes}
    order = sorted(classes, key=lambda c: -len(members[c]))

    perm = np.concatenate([members[c] for c in order])      # stream -> orig
    sz_of_stream = np.concatenate(
        [np.full(len(members[c]), len(members[c]), dtype=np.int64) for c in order]
    )
    cls_start = {}
    pos = 0
    for c in order:
        cls_start[c] = pos
        pos += len(members[c])

    x_s = x[perm]                                            # [N, F]
    sq = np.sum(x_s.astype(np.float32) * x_s, axis=1, dtype=np.float32)

    # Per-tier slot counts: M_t = max class size intersecting bins [8t, 8t+8)
    Ms = []
    for t in range(NB):
        lo, hi = 8 * t * BLK, 8 * (t + 1) * BLK
        Ms.append(int(sz_of_stream[lo:hi].max()))
    MW = max(Ms)

    # Host per-slot tensors in stream order.
    T = np.zeros((N, MW), dtype=np.float32)        # squared agree distances
    arank = np.zeros((N, MW), dtype=np.float32)    # A_j (agree rank, excl self)
    dcoef = np.zeros((N, MW), dtype=np.float32)    # mask*sqrt(T)/ (N-1)
    maskv = np.zeros((N, MW), dtype=np.float32)    # valid & not-self
    rcA = np.zeros((N, 1), dtype=np.float32)       # n_a*N (>=1)
    rcD = np.zeros((N, 1), dtype=np.float32)       # (N-sz)*N - N(N-1)/2

    for c in order:
        s = cls_start[c]
        sz = len(members[c])
        xc = x_s[s:s + sz]                                   # [sz, F]
        G = xc @ xc.T                                        # f32 gram
        sqc = sq[s:s + sz]
        D2 = sqc[:, None] + sqc[None, :] - 2.0 * G           # [sz, sz] f32
        # A[p, j] = #{l: D2[p, l] <= D2[p, j]} - 1   (remove self's count)
        A = (D2[:, None, :] <= D2[:, :, None]).sum(axis=2).astype(np.float32) - 1.0
        dist = np.sqrt(np.maximum(D2, 1e-12), dtype=np.float32)
        m = np.ones((sz, sz), dtype=np.float32)
        np.fill_diagonal(m, 0.0)
        T[s:s + sz, :sz] = D2
        arank[s:s + sz, :sz] = A * m                        # self slot -> 0
        dcoef[s:s + sz, :sz] = m * dist / np.float32(N - 1)
        maskv[s:s + sz, :sz] = m
        n_a = sz - 1
        rcA[s:s + sz, 0] = max(n_a * N, 1)
        rcD[s:s + sz, 0] = float((N - sz) * N - (N * (N - 1)) // 2)

    # Per-core gathers: core c rows = bins {c, 8+c, 16+c, 24+c} (t-major).
    core_rows = []
    for c in range(NCORES):
        rows = np.concatenate(
            [np.arange(128 * (8 * t + c), 128 * (8 * t + c) + 128) for t in range(NB)]
        )
        core_rows.append(rows)

    return dict(
        perm=perm, x_s=x_s, sq=sq, Ms=Ms, MW=MW,
        T=T, arank=arank, dcoef=dcoef, maskv=maskv, rcA=rcA, rcD=rcD,
        core_rows=core_rows,
    )


def _build_program(Ms, MW):
    import concourse.bacc as bacc
    import concourse.mybir as mybir
    import concourse.tile as tile

    dt = mybir.dt
    Alu = mybir.AluOpType

    nc = bacc.Bacc("TRN2")
    xT_d = nc.dram_tensor("xT", [F, N], dt.float32, kind="ExternalInput")
    sqone_d = nc.dram_tensor("sqone", [2, N], dt.float32, kind="ExternalInput")   # [sq; ones]
    xTL_d = nc.dram_tensor("xTL", [F, RPC], dt.float32, kind="ExternalInput")
    onesqL_d = nc.dram_tensor("onesqL", [2, RPC], dt.float32, kind="ExternalInput")  # [ones; sq_rows]
    T_d = nc.dram_tensor("T", [RPC, MW], dt.float32, kind="ExternalInput")
    ar_d = nc.dram_tensor("arank", [RPC, MW], dt.float32, kind="ExternalInput")
    dc_d = nc.dram_tensor("dcoef", [RPC, MW], dt.float32, kind="ExternalInput")
    mv_d = nc.dram_tensor("maskv", [RPC, MW], dt.float32, kind="ExternalInput")
    rcA_d = nc.dram_tensor("rcA", [RPC, 1], dt.float32, kind="ExternalInput")
    rcD_d = nc.dram_tensor("rcD", [RPC, 1], dt.float32, kind="ExternalInput")
    out_d = nc.dram_tensor("out", [BLK, NB], dt.float32, kind="ExternalOutput")

    # engine split: ACT gets slots [0, a), DVE gets [a, M)
    splits = []
    for M in Ms:
        a = int(round((4.48 * M + 3.0) / (4.48 + 3.9)))
        a = min(max(a, 0), M)
        splits.append(a)

    with tile.TileContext(nc) as tc:
        with (
            tc.tile_pool(name="big", bufs=1) as big,
            tc.tile_pool(name="inp", bufs=2) as inp,
            tc.tile_pool(name="sml", bufs=2) as sml,
            tc.tile_pool(name="ps", bufs=1, space="PSUM") as psp,
        ):
            xT = big.tile([F, N], dt.float32, tag="xT")
            nc.sync.dma_start(xT[:], xT_d[:])
            sqone = big.tile([2, N], dt.float32, tag="sqone")
            nc.sync.dma_start(sqone[:], sqone_d[:])
            xTL = big.tile([F, RPC], dt.float32, tag="xTL")
            nc.sync.dma_start(xTL[:], xTL_d[:])
            onesqL = big.tile([2, RPC], dt.float32, tag="onesqL")
            nc.sync.dma_start(onesqL[:], onesqL_d[:])
            junkD = big.tile([BLK, N], dt.float16, tag="junkD")
            junkA = big.tile([BLK, N], dt.float16, tag="junkA")
            out_sb = big.tile([BLK, NB], dt.float32, tag="outsb")

            for b in range(NB):
                M = Ms[b]
                a_split = splits[b]
                rlo = BLK * b

                # ---- D^2 block into PSUM: [128 rows x 4096] f32 ----
                ps = psp.tile([BLK, N], dt.float32, tag="ps")
                stage = sml.tile([F, BLK], dt.float32, tag="stage")
                nc.vector.tensor_scalar(
                    out=stage[:], in0=xTL[:, rlo:rlo + BLK], scalar1=-2.0,
                    scalar2=None, op0=Alu.mult)
                for tcol in range(N // 512):
                    cs = 512 * tcol
                    nc.tensor.matmul(ps[:, cs:cs + 512], stage[:],
                                     xT[:, cs:cs + 512], start=True, stop=False)
                    nc.tensor.matmul(ps[:, cs:cs + 512],
                                     onesqL[:, rlo:rlo + BLK],
                                     sqone[:, cs:cs + 512], start=False, stop=True)

                # ---- per-block inputs ----
                thr = inp.tile([BLK, M], dt.float32, tag="thr")
                nc.sync.dma_start(thr[:], T_d[rlo:rlo + BLK, 0:M])
                ar = inp.tile([BLK, M], dt.float32, tag="ar")
                nc.sync.dma_start(ar[:], ar_d[rlo:rlo + BLK, 0:M])
                dc = inp.tile([BLK, M], dt.float32, tag="dc")
                nc.sync.dma_start(dc[:], dc_d[rlo:rlo + BLK, 0:M])
                mv = inp.tile([BLK, M], dt.float32, tag="mv")
                nc.sync.dma_start(mv[:], mv_d[rlo:rlo + BLK, 0:M])
                rca = sml.tile([BLK, 1], dt.float32, tag="rca")
                nc.sync.dma_start(rca[:], rcA_d[rlo:rlo + BLK, :])
                rcd = sml.tile([BLK, 1], dt.float32, tag="rcd")
                nc.sync.dma_start(rcd[:], rcD_d[rlo:rlo + BLK, :])

                cnt = inp.tile([BLK, M], dt.float32, tag="cnt")
                sgn = inp.tile([BLK, M], dt.float32, tag="sgn")

                # ---- counts ----
                for j in range(a_split):      # ACT slots
                    nc.scalar.activation(
                        out=junkA[:], in_=ps[:],
                        func=mybir.ActivationFunctionType.Sign,
                        bias=thr[:, j:j + 1], scale=-1.0,
                        accum_out=sgn[:, j:j + 1])
                for j in range(a_split, M):   # DVE slots
                    nc.vector.tensor_scalar(
                        out=junkD[:], in0=ps[:], scalar1=thr[:, j:j + 1],
                        scalar2=0.0, op0=Alu.is_le, op1=Alu.add,
                        accum_out=cnt[:, j:j + 1])
                if a_split > 0:               # cnt = 2048 + sgn/2
                    nc.vector.tensor_scalar(
                        out=cnt[:, 0:a_split], in0=sgn[:, 0:a_split],
                        scalar1=0.5, scalar2=float(N // 2), op0=Alu.mult,
                        op1=Alu.add)

                # ---- epilogue ----
                tmp = inp.tile([BLK, M], dt.float32, tag="tmp")
                SR = sml.tile([BLK, 1], dt.float32, tag="SR")
                # SR = sum(maskv * (cnt - 1))
                nc.vector.scalar_tensor_tensor(
                    out=tmp[:], in0=cnt[:], scalar=-1.0, in1=mv[:],
                    op0=Alu.add, op1=Alu.mult, accum_out=SR[:])
                Sa = sml.tile([BLK, 1], dt.float32, tag="Sa")
                nc.vector.tensor_scalar(
                    out=Sa[:], in0=SR[:], scalar1=-1.0, scalar2=rca[:],
                    op0=Alu.mult, op1=Alu.add)
                Sd = sml.tile([BLK, 1], dt.float32, tag="Sd")
                nc.vector.tensor_scalar(
                    out=Sd[:], in0=SR[:], scalar1=1.0, scalar2=rcd[:],
                    op0=Alu.mult, op1=Alu.add)
                rSa = sml.tile([BLK, 1], dt.float32, tag="rSa")
                nc.vector.reciprocal(out=rSa[:], in_=Sa[:])
                rSd = sml.tile([BLK, 1], dt.float32, tag="rSd")
                nc.vector.reciprocal(out=rSd[:], in_=Sd[:])
                fa01 = inp.tile([BLK, M], dt.float32, tag="fa01")
                nc.vector.tensor_scalar(
                    out=fa01[:], in0=ar[:], scalar1=rSa[:], scalar2=0.1,
                    op0=Alu.mult, op1=Alu.add)
                rfa = inp.tile([BLK, M], dt.float32, tag="rfa")
                nc.vector.reciprocal(out=rfa[:], in_=fa01[:])
                B = inp.tile([BLK, M], dt.float32, tag="B")
                nc.vector.scalar_tensor_tensor(
                    out=B[:], in0=cnt[:], scalar=-1.0, in1=ar[:],
                    op0=Alu.add, op1=Alu.subtract)
                fd01 = inp.tile([BLK, M], dt.float32, tag="fd01")
                nc.vector.tensor_scalar(
                    out=fd01[:], in0=B[:], scalar1=rSd[:], scalar2=0.1,
                    op0=Alu.mult, op1=Alu.add)
                pr = inp.tile([BLK, M], dt.float32, tag="pr")
                nc.vector.tensor_tensor(
                    out=pr[:], in0=fd01[:], in1=rfa[:], op=Alu.mult)
                # score = sum(dcoef * pr)
                nc.vector.scalar_tensor_tensor(
                    out=tmp[:], in0=pr[:], scalar=1.0, in1=dc[:],
                    op0=Alu.mult, op1=Alu.mult,
                    accum_out=out_sb[:, b:b + 1])

            nc.sync.dma_start(out_d[:], out_sb[:])

    nc.compile()
    return nc


def kernel(x, y):
    from concourse.bass_utils import run_bass_kernel_spmd

    x = np.asarray(x, dtype=np.float32)
    y_in = np.asarray(y)
    lay = _host_layout(x, y_in)
    Ms, MW = lay["Ms"], lay["MW"]

    key = (tuple(Ms), MW)
    if key not in _cache:
        _cache[key] = _build_program(Ms, MW)
    nc = _cache[key]

    x_s, sq = lay["x_s"], lay["sq"]
    xT = np.ascontiguousarray(x_s.T)                         # [F, N]
    sqone = np.ascontiguousarray(
        np.stack([sq, np.ones(N, dtype=np.float32)]))        # [2, N]

    in_maps = []
    for c in range(NCORES):
        rows = lay["core_rows"][c]
        in_maps.append({
            "xT": xT,
            "sqone": sqone,
            "xTL": np.ascontiguousarray(x_s[rows].T),
            "onesqL": np.ascontiguousarray(
                np.stack([np.ones(RPC, dtype=np.float32), sq[rows]])),
            "T": np.ascontiguousarray(lay["T"][rows]),
            "arank": np.ascontiguousarray(lay["arank"][rows]),
            "dcoef": np.ascontiguousarray(lay["dcoef"][rows]),
            "maskv": np.ascontiguousarray(lay["maskv"][rows]),
            "rcA": np.ascontiguousarray(lay["rcA"][rows]),
            "rcD": np.ascontiguousarray(lay["rcD"][rows]),
        })

    globals()["_last"] = (nc, in_maps)
    res = run_bass_kernel_spmd(nc, in_maps, list(range(NCORES)))

    out_stream = np.zeros(N, dtype=np.float32)
    for c in range(NCORES):
        o = res.results[c]["out"]                            # [128, NB]
        rows = lay["core_rows"][c]
        for t in range(NB):
            out_stream[rows[BLK * t:BLK * (t + 1)]] = o[:, t]

    out = np.zeros(N, dtype=np.float32)
    out[lay["perm"]] = out_stream
    return out


# revision 6
# speedup vs baseline: 2.0138x; 2.0138x over previous
"""Trainium2 Bass kernel for nn_CDistLoss (retrieval_knn).

Math reduction (validated against the reference to ~3e-7 rel err):
  With MARGIN=0 the relu kills every disagree term, so
    out[i] = (1/(N-1)) * sum_{j in class(i), j!=i} D_ij * (0.1+fd_j)/(0.1+fa_j)
  where fa_j = A_j/S_a, fd_j = B_j/S_d, A_j = rank of j among same-class
  distances, B_j = R_j - A_j with R_j the global rank of D_ij in row i,
  S_a = n_a*N - sum_j R_j, S_d = n_d*N - N(N-1)/2 + sum_j R_j.
  The sample_performance/min/weight factor is 1.0 to ~4e-7 in f32 and is
  dropped.

Device work per row: the [N] row of squared distances (PE fp32 matmul into
PSUM) and one count-below-threshold per same-class member (DVE is_le+accum
and ACT Sign+accum instructions, split to balance both engines). Everything
that only touches the ~64 same-class values per row (thresholds, agree
ranks, score coefficients, masks) is precomputed on the host in f32 and fed
as input tensors, which also keeps the program identical across the 8 cores.

Rows are dealt to 32 bins of 128 in class-size-descending order; bin k runs
as block k//8 on core k%8, so every core executes the same static program
with per-tier slot counts M_t.
"""

import numpy as np

N = 4096
F = 128
NCORES = 8
RPC = 512          # rows per core
NB = 4             # blocks (tiers) per core
BLK = 128          # rows per block

_cache = {}


def _host_layout(x, y):
    """Class-sorted stream layout + all host-side per-slot tensors."""
    x = np.asarray(x, dtype=np.float32)
    y = np.asarray(y).astype(np.int64)

    classes, first_idx = np.unique(y, return_index=True)
    members = {c: np.where(y == c)[0] for c in classes}
    order = sorted(classes, key=lambda c: -len(members[c]))

    perm = np.concatenate([members[c] for c in order])      # stream -> orig
    sz_of_stream = np.concatenate(
        [np.full(len(members[c]), len(members[c]), dtype=np.int64) for c in order]
    )
    cls_start = {}
    pos = 0
    for c in order:
        cls_start[c] = pos
        pos += len(members[c])

    x_s = x[perm]                                            # [N, F]
    sq = np.sum(x_s.astype(np.float32) * x_s, axis=1, dtype=np.float32)

    # Per-tier slot counts: M_t = max class size intersecting bins [8t, 8t+8)
    Ms = []
    for t in range(NB):
        lo, hi = 8 * t * BLK, 8 * (t + 1) * BLK
        Ms.append(int(sz_of_stream[lo:hi].max()))
    MW = max(Ms)

    # Host per-slot tensors in stream order.
    T = np.zeros((N, MW), dtype=np.float32)        # squared agree distances
    arank = np.zeros((N, MW), dtype=np.float32)    # A_j (agree rank, excl self)
    dcoef = np.zeros((N, MW), dtype=np.float32)    # mask*sqrt(T)/ (N-1)
    maskv = np.zeros((N, MW), dtype=np.float32)    # valid & not-self
    rcA = np.zeros((N, 1), dtype=np.float32)       # n_a*N (>=1)
    rcD = np.zeros((N, 1), dtype=np.float32)       # (N-sz)*N - N(N-1)/2

    for c in order:
        s = cls_start[c]
        sz = len(members[c])
        xc = x_s[s:s + sz]                                   # [sz, F]
        G = xc @ xc.T                                        # f32 gram
        sqc = sq[s:s + sz]
        D2 = sqc[:, None] + sqc[None, :] - 2.0 * G           # [sz, sz] f32
        # A[p, j] = #{l: D2[p, l] <= D2[p, j]} - 1   (remove self's count)
        A = (D2[:, None, :] <= D2[:, :, None]).sum(axis=2).astype(np.float32) - 1.0
        dist = np.sqrt(np.maximum(D2, 1e-12), dtype=np.float32)
        m = np.ones((sz, sz), dtype=np.float32)
        np.fill_diagonal(m, 0.0)
        T[s:s + sz, :sz] = D2
        arank[s:s + sz, :sz] = A * m                        # self slot -> 0
        dcoef[s:s + sz, :sz] = m * dist / np.float32(N - 1)
        maskv[s:s + sz, :sz] = m
        n_a = sz - 1
        rcA[s:s + sz, 0] = max(n_a * N, 1)
        rcD[s:s + sz, 0] = float((N - sz) * N - (N * (N - 1)) // 2)

    # Per-core gathers: core c rows = bins {c, 8+c, 16+c, 24+c} (t-major).
    core_rows = []
    for c in range(NCORES):
        rows = np.concatenate(
            [np.arange(128 * (8 * t + c), 128 * (8 * t + c) + 128) for t in range(NB)]
        )
        core_rows.append(rows)

    return dict(
        perm=perm, x_s=x_s, sq=sq, Ms=Ms, MW=MW,
        T=T, arank=arank, dcoef=dcoef, maskv=maskv, rcA=rcA, rcD=rcD,
        core_rows=core_rows,
    )


def _build_program(Ms, MW):
    import concourse.bacc as bacc
    import concourse.mybir as mybir
    import concourse.tile as tile

    dt = mybir.dt
    Alu = mybir.AluOpType

    nc = bacc.Bacc("TRN2")
    xT_d = nc.dram_tensor("xT", [F, N], dt.float32, kind="ExternalInput")
    sqone_d = nc.dram_tensor("sqone", [2, N], dt.float32, kind="ExternalInput")   # [sq; ones]
    xTL_d = nc.dram_tensor("xTL", [F, RPC], dt.float32, kind="ExternalInput")
    onesqL_d = nc.dram_tensor("onesqL", [2, RPC], dt.float32, kind="ExternalInput")  # [ones; sq_rows]
    T_d = nc.dram_tensor("T", [RPC, MW], dt.float32, kind="ExternalInput")
    ar_d = nc.dram_tensor("arank", [RPC, MW], dt.float32, kind="ExternalInput")
    dc_d = nc.dram_tensor("dcoef", [RPC, MW], dt.float32, kind="ExternalInput")
    mv_d = nc.dram_tensor("maskv", [RPC, MW], dt.float32, kind="ExternalInput")
    rcA_d = nc.dram_tensor("rcA", [RPC, 1], dt.float32, kind="ExternalInput")
    rcD_d = nc.dram_tensor("rcD", [RPC, 1], dt.float32, kind="ExternalInput")
    out_d = nc.dram_tensor("out", [BLK, NB], dt.float32, kind="ExternalOutput")

    # engine split: ACT gets slots [0, a), DVE gets [a, M)
    # balance: a*3.86 + copies(5.8) = (M-a)*4.48 + epilogue(3.0)
    splits = []
    for M in Ms:
        a = int(round((4.48 * M - 2.8) / (4.48 + 3.86)))
        a = min(max(a, 0), M)
        splits.append(a)

    with tile.TileContext(nc) as tc:
        with (
            tc.tile_pool(name="big", bufs=1) as big,
            tc.tile_pool(name="inp", bufs=2) as inp,
            tc.tile_pool(name="sml", bufs=2) as sml,
            tc.tile_pool(name="ps", bufs=1, space="PSUM") as psp,
        ):
            xT = big.tile([F, N], dt.float32, tag="xT")
            nc.sync.dma_start(xT[:], xT_d[:])
            sqone = big.tile([2, N], dt.float32, tag="sqone")
            nc.sync.dma_start(sqone[:], sqone_d[:])
            xTL = big.tile([F, RPC], dt.float32, tag="xTL")
            nc.sync.dma_start(xTL[:], xTL_d[:])
            onesqL = big.tile([2, RPC], dt.float32, tag="onesqL")
            nc.sync.dma_start(onesqL[:], onesqL_d[:])
            junkD = big.tile([BLK, N], dt.float16, tag="junkD")
            junkA = big.tile([BLK, N], dt.float16, tag="junkA")
            out_sb = big.tile([BLK, NB], dt.float32, tag="outsb")

            for b in range(NB):
                M = Ms[b]
                a_split = splits[b]
                rlo = BLK * b

                # ---- D^2 block into PSUM: [128 rows x 4096] f32 ----
                ps = psp.tile([BLK, N], dt.float32, tag="ps")
                stage = sml.tile([F, BLK], dt.float32, tag="stage")
                nc.vector.tensor_scalar(
                    out=stage[:], in0=xTL[:, rlo:rlo + BLK], scalar1=-2.0,
                    scalar2=None, op0=Alu.mult)
                d2 = inp.tile([BLK, N], dt.float32, tag="d2")
                for tcol in range(N // 512):
                    cs = 512 * tcol
                    nc.tensor.matmul(ps[:, cs:cs + 512], stage[:],
                                     xT[:, cs:cs + 512], start=True, stop=False)
                    nc.tensor.matmul(ps[:, cs:cs + 512],
                                     onesqL[:, rlo:rlo + BLK],
                                     sqone[:, cs:cs + 512], start=False, stop=True)
                    # drain PSUM to SBUF so both count engines read SBUF
                    # (concurrent PSUM readers get serialized by bank deps)
                    nc.scalar.copy(d2[:, cs:cs + 512], ps[:, cs:cs + 512])

                # ---- per-block inputs ----
                thr = inp.tile([BLK, M], dt.float32, tag="thr")
                nc.sync.dma_start(thr[:], T_d[rlo:rlo + BLK, 0:M])
                ar = inp.tile([BLK, M], dt.float32, tag="ar")
                nc.sync.dma_start(ar[:], ar_d[rlo:rlo + BLK, 0:M])
                dc = inp.tile([BLK, M], dt.float32, tag="dc")
                nc.sync.dma_start(dc[:], dc_d[rlo:rlo + BLK, 0:M])
                mv = inp.tile([BLK, M], dt.float32, tag="mv")
                nc.sync.dma_start(mv[:], mv_d[rlo:rlo + BLK, 0:M])
                rca = sml.tile([BLK, 1], dt.float32, tag="rca")
                nc.sync.dma_start(rca[:], rcA_d[rlo:rlo + BLK, :])
                rcd = sml.tile([BLK, 1], dt.float32, tag="rcd")
                nc.sync.dma_start(rcd[:], rcD_d[rlo:rlo + BLK, :])

                cnt = inp.tile([BLK, M], dt.float32, tag="cnt")
                sgn = inp.tile([BLK, M], dt.float32, tag="sgn")

                # ---- counts ----
                for j in range(a_split):      # ACT slots
                    nc.scalar.activation(
                        out=junkA[:], in_=d2[:],
                        func=mybir.ActivationFunctionType.Sign,
                        bias=thr[:, j:j + 1], scale=-1.0,
                        accum_out=sgn[:, j:j + 1])
                for j in range(a_split, M):   # DVE slots
                    nc.vector.tensor_scalar(
                        out=junkD[:], in0=d2[:], scalar1=thr[:, j:j + 1],
                        scalar2=0.0, op0=Alu.is_le, op1=Alu.add,
                        accum_out=cnt[:, j:j + 1])
                if a_split > 0:               # cnt = 2048 + sgn/2
                    nc.vector.tensor_scalar(
                        out=cnt[:, 0:a_split], in0=sgn[:, 0:a_split],
                        scalar1=0.5, scalar2=float(N // 2), op0=Alu.mult,
                        op1=Alu.add)

                # ---- epilogue ----
                tmp = inp.tile([BLK, M], dt.float32, tag="tmp")
                SR = sml.tile([BLK, 1], dt.float32, tag="SR")
                # SR = sum(maskv * (cnt - 1))
                nc.vector.scalar_tensor_tensor(
                    out=tmp[:], in0=cnt[:], scalar=-1.0, in1=mv[:],
                    op0=Alu.add, op1=Alu.mult, accum_out=SR[:])
                Sa = sml.tile([BLK, 1], dt.float32, tag="Sa")
                nc.vector.tensor_scalar(
                    out=Sa[:], in0=SR[:], scalar1=-1.0, scalar2=rca[:],
                    op0=Alu.mult, op1=Alu.add)
                Sd = sml.tile([BLK, 1], dt.float32, tag="Sd")
                nc.vector.tensor_scalar(
                    out=Sd[:], in0=SR[:], scalar1=1.0, scalar2=rcd[:],
                    op0=Alu.mult, op1=Alu.add)
                rSa = sml.tile([BLK, 1], dt.float32, tag="rSa")
                nc.vector.reciprocal(out=rSa[:], in_=Sa[:])
                rSd = sml.tile([BLK, 1], dt.float32, tag="rSd")
                nc.vector.reciprocal(out=rSd[:], in_=Sd[:])
                fa01 = inp.tile([BLK, M], dt.float32, tag="fa01")
                nc.vector.tensor_scalar(
                    out=fa01[:], in0=ar[:], scalar1=rSa[:], scalar2=0.1,
                    op0=Alu.mult, op1=Alu.add)
                rfa = inp.tile([BLK, M], dt.float32, tag="rfa")
                nc.vector.reciprocal(out=rfa[:], in_=fa01[:])
                B = inp.tile([BLK, M], dt.float32, tag="B")
                nc.vector.scalar_tensor_tensor(
                    out=B[:], in0=cnt[:], scalar=-1.0, in1=ar[:],
                    op0=Alu.add, op1=Alu.subtract)
                fd01 = inp.tile([BLK, M], dt.float32, tag="fd01")
                nc.vector.tensor_scalar(
                    out=fd01[:], in0=B[:], scalar1=rSd[:], scalar2=0.1,
                    op0=Alu.mult, op1=Alu.add)
                pr = inp.tile([BLK, M], dt.float32, tag="pr")
                nc.vector.tensor_tensor(
                    out=pr[:], in0=fd01[:], in1=rfa[:], op=Alu.mult)
                # score = sum(dcoef * pr)
                nc.vector.scalar_tensor_tensor(
                    out=tmp[:], in0=pr[:], scalar=1.0, in1=dc[:],
                    op0=Alu.mult, op1=Alu.mult,
                    accum_out=out_sb[:, b:b + 1])

            nc.sync.dma_start(out_d[:], out_sb[:])

    nc.compile()
    return nc


def kernel(x, y):
    from concourse.bass_utils import run_bass_kernel_spmd

    x = np.asarray(x, dtype=np.float32)
    y_in = np.asarray(y)
    lay = _host_layout(x, y_in)
    Ms, MW = lay["Ms"], lay["MW"]

    key = (tuple(Ms), MW)
    if key not in _cache:
        _cache[key] = _build_program(Ms, MW)
    nc = _cache[key]

    x_s, sq = lay["x_s"], lay["sq"]
    xT = np.ascontiguousarray(x_s.T)                         # [F, N]
    sqone = np.ascontiguousarray(
        np.stack([sq, np.ones(N, dtype=np.float32)]))        # [2, N]

    in_maps = []
    for c in range(NCORES):
        rows = lay["core_rows"][c]
        in_maps.append({
            "xT": xT,
            "sqone": sqone,
            "xTL": np.ascontiguousarray(x_s[rows].T),
            "onesqL": np.ascontiguousarray(
                np.stack([np.ones(RPC, dtype=np.float32), sq[rows]])),
            "T": np.ascontiguousarray(lay["T"][rows]),
            "arank": np.ascontiguousarray(lay["arank"][rows]),
            "dcoef": np.ascontiguousarray(lay["dcoef"][rows]),
            "maskv": np.ascontiguousarray(lay["maskv"][rows]),
            "rcA": np.ascontiguousarray(lay["rcA"][rows]),
            "rcD": np.ascontiguousarray(lay["rcD"][rows]),
        })

    globals()["_last"] = (nc, in_maps)
    res = run_bass_kernel_spmd(nc, in_maps, list(range(NCORES)))

    out_stream = np.zeros(N, dtype=np.float32)
    for c in range(NCORES):
        o = res.results[c]["out"]                            # [128, NB]
        rows = lay["core_rows"][c]
        for t in range(NB):
            out_stream[rows[BLK * t:BLK * (t + 1)]] = o[:, t]

    out = np.zeros(N, dtype=np.float32)
    out[lay["perm"]] = out_stream
    return out
